# revision 22
# baseline (speedup 1.0000x reference)
"""AtomicBasis GNN message-passing kernel for 8 TRN2 NeuronCores (v2).

A[k,x,y,z] = sum_a  c*sin(k*pi*d_a/5)/d_a * (h@W.T)[a,k] * nx*ny*nz
with n = rel_pos/d.  Rewritten as  A = sum_a w[a,k] * m[a,s] where
  w[a,k] = sin(2*pi*frac(k*d_a/10)) * (h@W.T)[a,k]
  m[a,s] = monomial_s(rp) * c/d^4     (s = 10 distinct symmetric monomials)
Shard a across 8 cores (data parallel); sum the (128,20) partials on host.

v2 engine plan (per core, Q=992 q's per partition):
  - phases theta16 = round(d/10 * 2^16) as u16.
  - q in [0,448): GPSIMD argu path: u32 wrapping broadcast mult
    (theta32 = theta16<<16 times k wraps mod 2^32 = free range reduce),
    ACT Sin from i32 bitcast.  k-fast layout.
  - q in [448,992): DVE argu path: per-k fused tensor_scalar
    t = thf*(k/2^16) + 1.5*2^7  (one legal arith pair; MOD/int ops are
    forbidden in DVE TensorScalar).  For y = k*theta_turns < 64 the f32
    bit pattern of t has ulp 2^-16, so its LOW 16 BITS are exactly
    frac(y)*2^16.  The Sin ACT reads those bits as int16 via a stride-2
    byte-view (free range reduction) and writes sinb k-fast via a
    strided out AP (free transpose).  Requires d < 10 (holds: d <= ~7).
  - fold w = sin*hp in 8-pair blocks (1024 elems): mix of
    direct DVE (PSUM read, 1x), ACT-copy->SBUF-bf16 + DVE 2x, and
    ACT-copy + GPSIMD fold, pattern-tuned to balance engines.
  - MMW: PE matmul lhsT=hT-pair (pair-stacked host layout), rhs=blkdiag W.
    MMA: PE matmul lhsT=w-pair, rhs=m10-pair accumulated in one PSUM bank.
  - c/d^4 via ACT Ln+Exp (no DVE reciprocal).
"""

import os
import sys
import numpy as np

for _p in ("/opt/trn_rl_repo", "/root/problem/trn_rl_repo"):
    if os.path.isdir(_p) and _p not in sys.path:
        sys.path.insert(0, _p)

import ml_dtypes

N_GLOBAL = 1_000_000
K = 64
P = 128
Q = 992                      # q's (a-columns) per partition per core
NLOC = P * Q                 # 126976 per core
NCORES = 8
NTOT = NCORES * NLOC         # 1015808 >= 1e6 (padded)
R_CUT = 5.0
C_RBF = float(np.sqrt(2.0 / R_CUT))

QSPLIT = 448                 # q's on the GPSIMD-argu path (rest: DVE path)
QH1 = Q - QSPLIT             # 544
NCH0 = QSPLIT // 32          # 14 GP-argu chunks of 32 q
NOCT = Q // 16               # 62 fold blocks ("octs") of 8 pairs = 16 q
NGRAN = Q // 32              # 31 hT DMA granules of 16 pairs
FIX16 = 65536.0
SC16 = float((2.0 ** 16 / (2.0 * R_CUT)) ** 2)   # thf = sqrt(d^2*SC16) = d/10*2^16

# fold path per oct index: 'x' direct DVE, 'z' ACT-copy + DVE 2x,
# 'y' ACT-copy + GPSIMD fold.  z requires packed (k-fast) sinb -> H0 only.
FOLD_PATTERN_H0 = "zxzyzxz"
FOLD_PATTERN_H1 = "xyxxxy"

# s-index -> monomial: s = 3*alpha+beta is rp[alpha]^2*rp[beta]*q2 (s 0..8),
# s=9 is x*y*z*q2. Host expands 10 -> 27 via sorted-multiset lookup.
_MONO = {}
for _a in range(3):
    for _b in range(3):
        _MONO.setdefault(tuple(sorted([_a, _a, _b])), 3 * _a + _b)
_MONO[(0, 1, 2)] = 9

_CACHE = {}


def _build_nc():
    import concourse.bass as bass
    import concourse.bacc as bacc
    import concourse.tile as tile
    import concourse.mybir as mybir

    f32 = mybir.dt.float32
    bf16 = mybir.dt.bfloat16
    u32 = mybir.dt.uint32
    i32 = mybir.dt.int32
    u16 = mybir.dt.uint16
    i16 = mybir.dt.int16

    nc = bacc.Bacc(
        "TRN2",
        target_bir_lowering=False,
        debug=False,
        enable_asserts=True,
        num_devices=NCORES,
    )

    HT_COLS = (Q // 2) * P
    htp_ext = nc.dram_tensor("htp", [P, HT_COLS], bf16, kind="ExternalInput").ap()
    rp_ext = nc.dram_tensor("rp", [3, NLOC], f32, kind="ExternalInput").ap()
    blkw_ext = nc.dram_tensor("blkw", [P, P], bf16, kind="ExternalInput").ap()
    io_ext = nc.dram_tensor("iou", [P, K], u32, kind="ExternalInput").ap()
    out_ext = nc.dram_tensor("out", [P, 20], f32, kind="ExternalOutput").ap()

    SIN = mybir.ActivationFunctionType.Sin
    SQRT = mybir.ActivationFunctionType.Sqrt
    LN = mybir.ActivationFunctionType.Ln
    EXP = mybir.ActivationFunctionType.Exp
    COPY = mybir.ActivationFunctionType.Copy
    MULT = mybir.AluOpType.mult
    ADD = mybir.AluOpType.add

    with tile.TileContext(nc) as tc:
        from contextlib import ExitStack

        with ExitStack() as ctx:
            # ---- persistent pool (prologue outputs + consts) ----
            const = ctx.enter_context(tc.tile_pool(name="const", bufs=1))
            blkw = const.tile([P, P], bf16)
            iou = const.tile([P, K], u32)
            zcol = const.tile([P, 1], f32)
            lncol = const.tile([P, 1], f32)
            thf = const.tile([P, Q], f32)               # theta_turns * 2^16
            thu32 = const.tile([P, QSPLIT], u32)
            m10 = const.tile([P, 10 * Q], bf16)
            m10v = m10[:].rearrange("p (s q) -> p s q", s=10)
            sinb0 = const.tile([P, QSPLIT * K], bf16)   # k-fast: col = q*64+k
            sinb1 = const.tile([P, QH1 * K], bf16)      # k-fast, q local to H1

            nc.sync.dma_start(blkw[:], blkw_ext)
            nc.sync.dma_start(iou[:], io_ext)
            nc.vector.memset(zcol[:], 0.0)
            nc.vector.memset(lncol[:], float(np.log(C_RBF)))

            # ---- prologue (scoped transients) ----
            with tc.tile_pool(name="prol", bufs=1) as prol:
                rp_all = prol.tile([P, 3 * Q], f32)
                nc.sync.dma_start(
                    rp_all[:].rearrange("p (x q) -> p x q", x=3),
                    rp_ext.rearrange("x (p q) -> p x q", p=P),
                )
                rx = rp_all[:, 0 * Q : 1 * Q]
                ry = rp_all[:, 1 * Q : 2 * Q]
                rz = rp_all[:, 2 * Q : 3 * Q]
                rp3 = rp_all[:].rearrange("p (x q) -> p x q", x=3)

                t_a = prol.tile([P, Q], f32)
                t_b = prol.tile([P, Q], f32)
                d2 = prol.tile([P, Q], f32)
                nc.vector.tensor_mul(t_a[:], rx, rx)
                nc.vector.tensor_mul(t_b[:], ry, ry)
                nc.vector.tensor_add(t_a[:], t_a[:], t_b[:])
                nc.vector.tensor_mul(t_b[:], rz, rz)
                nc.vector.tensor_add(d2[:], t_a[:], t_b[:])

                # thf = d/10 * 2^16 (as f32); thu32 = theta_turns*2^32 as u32
                nc.scalar.activation(thf[:], d2[:], SQRT, bias=zcol[:], scale=SC16)
                nc.scalar.activation(
                    thu32[:], thf[:, 0:QSPLIT], COPY, bias=0.0, scale=FIX16
                )

                # q2 = c/d^4 = exp(-2*ln(d2) + ln(c)); reuse t_a/t_b as scratch
                lnq = t_a
                nc.scalar.activation(lnq[:], d2[:], LN, bias=zcol[:], scale=1.0)
                q2 = t_b
                nc.scalar.activation(
                    q2[:], lnq[:], EXP, bias=lncol[:], scale=-2.0
                )

                rp_s = prol.tile([P, 3 * Q], f32)        # rp * (c/d^4)
                rps3 = rp_s[:].rearrange("p (x q) -> p x q", x=3)
                nc.vector.tensor_mul(
                    rps3, rp3, q2[:].unsqueeze(1).broadcast_to((P, 3, Q))
                )
                sq_s = prol.tile([P, 3 * Q], f32)        # rp^2 * (c/d^4)
                sqs3 = sq_s[:].rearrange("p (x q) -> p x q", x=3)
                nc.gpsimd.tensor_mul(sqs3, rp3, rps3)
                xyq = d2                                 # x*y*(c/d^4), reuse d2
                nc.vector.tensor_mul(xyq[:], rx, rp_s[:, 1 * Q : 2 * Q])

                # m10: 10 plain (P,Q) muls, split DVE/GPSIMD
                for s in range(9):
                    al, be = divmod(s, 3)
                    eng = nc.gpsimd if s % 2 == 0 else nc.vector
                    eng.tensor_mul(
                        m10[:, s * Q : (s + 1) * Q],
                        sq_s[:, al * Q : (al + 1) * Q],
                        rp_all[:, be * Q : (be + 1) * Q],
                    )
                nc.vector.tensor_mul(m10[:, 9 * Q : 10 * Q], xyq[:], rz)

            # ---- main pools ----
            hpool = ctx.enter_context(tc.tile_pool(name="hch", bufs=3))
            agp = ctx.enter_context(tc.tile_pool(name="agp", bufs=2))
            adv = ctx.enter_context(tc.tile_pool(name="adv", bufs=2))
            wpool = ctx.enter_context(tc.tile_pool(name="wf", bufs=2))
            hsb = ctx.enter_context(tc.tile_pool(name="hsb", bufs=2))
            php = ctx.enter_context(
                tc.tile_pool(name="php", bufs=3, space=bass.MemorySpace.PSUM)
            )
            psA = ctx.enter_context(
                tc.tile_pool(name="psA", bufs=1, space=bass.MemorySpace.PSUM)
            )

            A_ps = psA.tile([P, 20], f32)

            hT_tiles = {}

            def ensure_granule(g):
                if g not in hT_tiles:
                    t = hpool.tile([P, 2048], bf16, tag="hT")
                    nc.sync.dma_start(
                        t[:], htp_ext[:, g * 2048 : (g + 1) * 2048]
                    )
                    hT_tiles[g] = t
                return hT_tiles[g]

            state = {"hp": None, "oct": -1}

            def mmw_oct(o):
                """Issue 8 MMW matmuls for oct o into a fresh PSUM tile."""
                gran = ensure_granule(o // 2)
                base = (o % 2) * 8
                hp = php.tile([P, 1024], f32, tag="hp")
                for t in range(8):
                    nc.tensor.matmul(
                        hp[:, 128 * t : 128 * (t + 1)],
                        gran[:, 128 * (base + t) : 128 * (base + t + 1)],
                        blkw[:],
                        start=True,
                        stop=True,
                        skip_group_check=True,
                    )
                state["hp"] = hp
                state["oct"] = o

            def fold_mma_oct(o, hp):
                """Fold sinb*hp -> w (path by pattern) and issue 8 MMAs."""
                if o < 2 * NCH0:
                    sv = sinb0[:, (o * 16) * K : (o * 16 + 16) * K]
                    pat = FOLD_PATTERN_H0
                else:
                    # k-slow sinb1: strided read (q fast within oct, k slow)
                    q0l = o * 16 - QSPLIT
                    sv = sinb1[:].rearrange("p (k q) -> p q k", q=QH1)[
                        :, q0l : q0l + 16, :
                    ]
                    pat = FOLD_PATTERN_H1
                path = pat[o % len(pat)]
                w = wpool.tile([P, 1024], bf16, tag="w")
                if path == "x":
                    nc.vector.tensor_mul(w[:], sv, hp[:])
                else:
                    hs = hsb.tile([P, 1024], bf16, tag="hs")
                    nc.scalar.activation(hs[:], hp[:], COPY, bias=0.0, scale=1.0)
                    eng = nc.vector if path == "z" else nc.gpsimd
                    eng.tensor_mul(w[:], sv, hs[:])
                for t in range(8):
                    gp = 8 * o + t
                    nc.tensor.matmul(
                        A_ps[:],
                        w[:, 128 * t : 128 * (t + 1)],
                        m10v[:, :, 2 * gp : 2 * gp + 2],
                        start=(gp == 0),
                        stop=(gp == 8 * NOCT - 1),
                        skip_group_check=True,
                    )

            def pipelined_oct(o):
                """Software pipeline: MMW for oct o, fold+MMA for oct o-1."""
                prev_hp, prev_o = state["hp"], state["oct"]
                mmw_oct(o)
                if prev_o >= 0:
                    fold_mma_oct(prev_o, prev_hp)

            # DVE-argu groups for H1: 16 groups of 4 k's.
            # t = thf*(k/2^16) + 192.0 puts frac(k*theta_turns)*2^16 exactly
            # in the low 16 bits of the f32 word (t in [192,256) => ulp 2^-16).
            def dve_argu_group(g):
                ag = adv.tile([P, 4 * QH1], f32, tag="adv")
                for j in range(4):
                    k = 4 * g + j + 1
                    nc.vector.tensor_scalar(
                        ag[:, j * QH1 : (j + 1) * QH1],
                        thf[:, QSPLIT:Q],
                        float(k) / FIX16,
                        192.0,
                        MULT,
                        ADD,
                    )
                # sin from the low-halfword bits; contiguous write into
                # sinb1 K-SLOW layout (col = k*QH1 + q).  Strided ACT writes
                # measured 5x slow on HW, so folds eat the strided read
                # instead (they are 1x regardless).
                nc.scalar.activation(
                    sinb1[:, (4 * g) * QH1 : (4 * g + 4) * QH1],
                    ag[:]
                    .bitcast(i16)
                    .rearrange("p (n t) -> p n t", t=2)[:, :, 0],
                    SIN,
                    bias=zcol[:],
                    scale=float(2.0 * np.pi / FIX16),
                )

            # ---- H0: GP-argu chunks + interleaved work ----
            for c in range(NCH0):
                a0 = agp.tile([P, 2048], u32, tag="agp")
                nc.gpsimd.tensor_mul(
                    a0[:].rearrange("p (i k) -> p i k", i=32),
                    thu32[:, c * 32 : (c + 1) * 32]
                    .unsqueeze(2)
                    .broadcast_to((P, 32, K)),
                    iou[:].unsqueeze(1).broadcast_to((P, 32, K)),
                )
                nc.scalar.activation(
                    sinb0[:, c * 2048 : (c + 1) * 2048],
                    a0[:].bitcast(i32),
                    SIN,
                    bias=zcol[:],
                    scale=float(2.0 * np.pi / (FIX16 * FIX16)),
                )
                dve_argu_group(c)
                if c >= 12:
                    dve_argu_group(c + 2)
                pipelined_oct(2 * c)
                pipelined_oct(2 * c + 1)

            # ---- H1 octs ----
            for o in range(2 * NCH0, NOCT):
                pipelined_oct(o)
            fold_mma_oct(state["oct"], state["hp"])

            # ---- epilogue ----
            A_sb = const.tile([P, 20], f32)
            nc.vector.tensor_copy(A_sb[:], A_ps[:])
            nc.gpsimd.dma_start(out_ext, A_sb[:])

    nc.compile()
    return nc


def _get_nc():
    if "nc" not in _CACHE:
        _CACHE["nc"] = _build_nc()
    return _CACHE["nc"]


def kernel(h, rel_poss, W):
    from concourse.bass_utils import run_bass_kernel_spmd

    nc = _get_nc()

    h_pad = np.zeros((NTOT, K), dtype=np.float32)
    h_pad[:N_GLOBAL] = h
    rp_pad = np.ones((3, NTOT), dtype=np.float32)
    rp_pad[:, :N_GLOBAL] = rel_poss

    # Pre-transpose h to pair-stacked bf16 layout:
    # htp[i, 64*o + j, pi*128 + c] = h[i*NLOC + c*Q + 2*pi + o, j]
    Hc = h_pad.reshape(NCORES, P, Q, K).astype(ml_dtypes.bfloat16)
    ht = Hc.transpose(0, 3, 2, 1)                     # (i, j, q, c)
    htp = np.ascontiguousarray(
        ht.reshape(NCORES, K, Q // 2, 2, P).transpose(0, 3, 1, 2, 4)
    ).reshape(NCORES, P, (Q // 2) * P)

    wt = np.ascontiguousarray(W.T.astype(np.float32))   # wt[j,k] = W[k,j]
    blkw = np.zeros((P, P), dtype=np.float32)
    blkw[0:K, 0:K] = wt
    blkw[K:P, K:P] = wt
    blkw = blkw.astype(ml_dtypes.bfloat16)

    iou = np.ascontiguousarray(
        np.broadcast_to(np.arange(1, K + 1, dtype=np.uint32), (P, K))
    )

    in_maps = []
    for i in range(NCORES):
        in_maps.append(
            {
                "htp": htp[i],
                "rp": np.ascontiguousarray(rp_pad[:, i * NLOC : (i + 1) * NLOC]),
                "blkw": blkw,
                "iou": iou,
            }
        )

    res = run_bass_kernel_spmd(
        nc, in_maps, core_ids=list(range(NCORES)), trace=_CACHE.get("trace", False)
    )
    _CACHE["last_results"] = res
    acc = np.sum(
        [np.asarray(res.results[i]["out"], dtype=np.float32) for i in range(NCORES)],
        axis=0,
    )                                               # (128, 20)
    a20 = acc.reshape(P, 10, 2)
    A10 = a20[0:K, :, 0] + a20[K:P, :, 1]           # (64, 10)

    A = np.empty((K, 3, 3, 3), dtype=np.float32)
    for x in range(3):
        for y in range(3):
            for z in range(3):
                A[:, x, y, z] = A10[:, _MONO[tuple(sorted((x, y, z)))]]
    return A


if __name__ == "__main__":
    nc = _get_nc()
    print("build + compile OK")
